# revision 1
# baseline (speedup 1.0000x reference)
"""BiGN (2-relation LightGCN-style GNN) on 8 Trainium2 NeuronCores.

Strategy (dst-sharded, SPMD):
- Node space padded to 8 x 18816 rows; core k owns rows [k*18816, (k+1)*18816).
- Edges of both relations are routed to the dst-owner core, grouped by
  (8-block super-tile, src 32K chunk, 128-row dst block), padded to x128.
- Per layer: dma_gather rows of the replicated table (HBM) by src index,
  scale by edge value (DVE, bf16), build one-hot dst masks via iota/is_equal
  (DVE, bf16), segment-sum via PE matmul into PSUM, then the dual-relation
  attention combine (DVE/ACT), all per super-tile.
- New embeddings are AllGather'd across the 8 cores into the next layer's
  replicated table. Final layer mean -> per-core light_out slice; host
  assembles and does the tiny batch dot.
"""

import os
import numpy as np
import ml_dtypes

# ---------------------------------------------------------------- constants
NCORES = 8
N_USER = 100000
N_ITEM = 50000
N = N_USER + N_ITEM
D = 64
LAYERS = 3
NODES_PER_CORE_REAL = N // NCORES        # 18750
BLOCKS_PER_CORE = 147
NPC = BLOCKS_PER_CORE * 128              # 18816
NPAD = NCORES * NPC                      # 150528
CHUNK = 32768
NCHUNKS = (NPAD + CHUNK - 1) // CHUNK    # 5
STILE_BLOCKS = 8
NSTILES = (BLOCKS_PER_CORE + STILE_BLOCKS - 1) // STILE_BLOCKS  # 19
PAD_DPOS = 200.0
BF16 = ml_dtypes.bfloat16

_CACHE = {}


# ---------------------------------------------------------------- host prep
def _remap(n):
    return (n // NODES_PER_CORE_REAL) * NPC + (n % NODES_PER_CORE_REAL)


def _preprocess(graph_src, graph_dst, graph_val, sim_src, sim_dst, sim_val):
    sets = []
    for (s, d, v) in ((graph_src, graph_dst, graph_val),
                      (sim_src, sim_dst, sim_val)):
        sp = _remap(s.astype(np.int64))
        dp = _remap(d.astype(np.int64))
        sets.append(dict(core=dp // NPC, blk=(dp % NPC) // 128,
                         dpos=dp % 128, chunk=sp // CHUNK,
                         lsrc=sp % CHUNK, val=v))

    counts = np.zeros((NCORES, 2, BLOCKS_PER_CORE, NCHUNKS), np.int64)
    for si, e in enumerate(sets):
        np.add.at(counts, (e["core"], si, e["blk"], e["chunk"]), 1)
    n_mm = np.maximum(1, -(-counts.max(axis=0) // 128))  # [2, BLK, NCH]

    streams = []
    meta = None
    for c in range(NCORES):
        gidx_parts, val_parts, dpos_parts = [], [], []
        mmeta = []
        sel = {}
        for si, e in enumerate(sets):
            m = e["core"] == c
            key = e["blk"][m].astype(np.int64) * NCHUNKS + e["chunk"][m]
            order = np.argsort(key, kind="stable")
            idx_sorted = np.nonzero(m)[0][order]
            key_sorted = key[order]
            starts = np.searchsorted(
                key_sorted, np.arange(BLOCKS_PER_CORE * NCHUNKS))
            ends = np.searchsorted(
                key_sorted, np.arange(BLOCKS_PER_CORE * NCHUNKS) + 1)
            sel[si] = (idx_sorted, starts, ends)

        for st in range(NSTILES):
            blocks = range(st * STILE_BLOCKS,
                           min((st + 1) * STILE_BLOCKS, BLOCKS_PER_CORE))
            for si, e in enumerate(sets):
                idx_sorted, starts, ends = sel[si]
                for ch in range(NCHUNKS):
                    gi_l, va_l, dp_l = [], [], []
                    for b in blocks:
                        k = b * NCHUNKS + ch
                        eidx = idx_sorted[starts[k]:ends[k]]
                        L = n_mm[si, b, ch] * 128
                        gi = np.zeros(L, np.int16)
                        va = np.zeros(L, np.float32)
                        dpz = np.full(L, PAD_DPOS, np.float32)
                        gi[:len(eidx)] = e["lsrc"][eidx]
                        va[:len(eidx)] = e["val"][eidx]
                        dpz[:len(eidx)] = e["dpos"][eidx]
                        gi_l.append(gi); va_l.append(va); dp_l.append(dpz)
                    gi = np.concatenate(gi_l)
                    va = np.concatenate(va_l)
                    dpz = np.concatenate(dp_l)
                    if c == 0:
                        mmeta.append((st, si, ch, len(gi)))
                    gw = np.ascontiguousarray(gi.reshape(-1, 16).T)
                    gidx_parts.append(np.tile(gw, (8, 1)))
                    val_parts.append(np.ascontiguousarray(
                        va.reshape(-1, 128).T))
                    dpos_parts.append(np.ascontiguousarray(
                        dpz.reshape(-1, 128).T))
        streams.append(dict(
            gidx=np.concatenate(gidx_parts, axis=1),
            val=np.concatenate(val_parts, axis=1),
            dpos=np.concatenate(dpos_parts, axis=1).astype(BF16),
        ))
        if c == 0:
            meta = mmeta
    return n_mm, meta, streams


# ---------------------------------------------------------------- device
def _build_module(n_mm, meta, tot16, tot128):
    import concourse.bacc as bacc
    import concourse.mybir as mybir
    import concourse.tile as tile
    from concourse.library_config import mlp

    f32 = mybir.dt.float32
    bf16 = mybir.dt.bfloat16

    nc = bacc.Bacc("TRN2", target_bir_lowering=False, debug=False,
                   num_devices=NCORES, num_swdge_queues=4)
    emb_slice = nc.dram_tensor("emb_slice", [NPC, D], f32,
                               kind="ExternalInput")
    gidx = nc.dram_tensor("gidx", [128, tot16], mybir.dt.int16,
                          kind="ExternalInput")
    val_in = nc.dram_tensor("val", [128, tot128], f32, kind="ExternalInput")
    dpos_in = nc.dram_tensor("dpos", [128, tot128], bf16,
                             kind="ExternalInput")
    light_out = nc.dram_tensor("light_out", [NPC, D], f32,
                               kind="ExternalOutput")

    # offsets per call in the concatenated streams
    offs = []
    o16 = o128 = 0
    for (st, si, ch, L) in meta:
        offs.append((o16, o128))
        o16 += L // 16
        o128 += L // 128
    assert o16 == tot16 and o128 == tot128
    call_of = {}
    for i, (st, si, ch, L) in enumerate(meta):
        call_of[(st, si, ch)] = (L,) + offs[i]

    with tile.TileContext(nc) as tc:
        nc.gpsimd.load_library(mlp)
        with (
            tc.tile_pool(name="persist", bufs=1) as pers,
            tc.tile_pool(name="gath", bufs=4) as gpool,
            tc.tile_pool(name="small", bufs=8) as spool,
            tc.tile_pool(name="att", bufs=2) as apool,
            tc.tile_pool(name="psum", bufs=4, space="PSUM") as ppool,
            tc.tile_pool(name="dram", bufs=1, space="DRAM") as dram,
            tc.tile_pool(name="dstage", bufs=2, space="DRAM") as dstage,
        ):
            iota_t = pers.tile([128, 128], bf16)
            nc.gpsimd.iota(iota_t[:], [[1, 128]], channel_multiplier=0,
                           allow_small_or_imprecise_dtypes=True)
            light = pers.tile([128, BLOCKS_PER_CORE, D], f32)
            emb_own = pers.tile([128, BLOCKS_PER_CORE, D], f32)
            # emb_own[p, b, :] = emb rows (b*128+p) of this core's slice
            nc.sync.dma_start(
                emb_own[:],
                emb_slice[:].rearrange("(b p) d -> p b d", p=128))

            tables = [dram.tile([NPAD, D], f32, addr_space="Shared",
                                name=f"table{i}") for i in range(LAYERS)]
            stage0 = dstage.tile([NPC, D], f32, tag="stage")
            nc.sync.dma_start(stage0[:], emb_slice[:])
            nc.gpsimd.collective_compute(
                "AllGather", mybir.AluOpType.bypass,
                ins=[stage0.opt()], outs=[tables[0].opt()],
                replica_groups=[list(range(NCORES))])

            gather_q = 0  # round-robin SWDGE queues -> 4 Q7 core pairs
            for layer in range(LAYERS):
                src_table = tables[layer]
                stg = None
                if layer < LAYERS - 1:
                    stg = dstage.tile([NPC, D], f32, tag="stage",
                                      name=f"stage{layer}")
                for st in range(NSTILES):
                    blocks = list(range(
                        st * STILE_BLOCKS,
                        min((st + 1) * STILE_BLOCKS, BLOCKS_PER_CORE)))
                    nblk = len(blocks)
                    ps = {}
                    for si in range(2):
                        ps[si] = ppool.tile([128, STILE_BLOCKS, D], f32,
                                            tag=f"ps{si}", name=f"ps{si}")
                        # zero data; all MMs run start=False. has_written
                        # per element: set -> add onto 0, clear -> overwrite
                        # with own value. Correct either way, order-free.
                        # (start=True would clear has_written for the WHOLE
                        # bank, wiping sibling blocks' accumulation state.)
                        nc.vector.memset(ps[si][:], 0.0)
                        for ch in range(NCHUNKS):
                            L, co16, co128 = call_of[(st, si, ch)]
                            M = L // 128
                            it = spool.tile([128, L // 16], mybir.dt.int16,
                                            tag="gidx")
                            nc.sync.dma_start(
                                it[:], gidx[:, co16:co16 + L // 16])
                            gt = gpool.tile([128, M, D], f32, tag="gath")
                            cbase = ch * CHUNK
                            cend = min(cbase + CHUNK, NPAD)
                            nc.gpsimd.dma_gather(
                                gt[:], src_table[cbase:cend, :], it[:],
                                L, L, D, single_packet=False,
                                queue_num=gather_q)
                            gather_q = (gather_q + 1) % 4
                            vt = spool.tile([128, M], f32, tag="val")
                            nc.sync.dma_start(
                                vt[:], val_in[:, co128:co128 + M])
                            dt = spool.tile([128, M], bf16, tag="dpos")
                            nc.sync.dma_start(
                                dt[:], dpos_in[:, co128:co128 + M])
                            ms = gpool.tile([128, M, D], bf16, tag="msgs")
                            nc.vector.tensor_tensor(
                                ms[:], gt[:],
                                vt[:].unsqueeze(2).to_broadcast([128, M, D]),
                                mybir.AluOpType.mult)
                            oh = gpool.tile([128, M, 128], bf16, tag="oh")
                            nc.vector.tensor_tensor(
                                oh[:],
                                dt[:].unsqueeze(2).to_broadcast([128, M, 128]),
                                iota_t[:].unsqueeze(1).to_broadcast(
                                    [128, M, 128]),
                                mybir.AluOpType.is_equal)
                            m = 0
                            for bl, b in enumerate(blocks):
                                for k in range(n_mm[si, b, ch]):
                                    nc.tensor.matmul(
                                        ps[si][:, bl, :],
                                        oh[:, m, :], ms[:, m, :],
                                        start=False, stop=False,
                                        skip_group_check=True)
                                    m += 1
                            assert m == M

                    # ---- attention / combine for this super-tile
                    sl = (slice(None), slice(0, nblk), slice(None))
                    eo = emb_own[:, blocks[0]:blocks[0] + nblk, :]
                    e1 = apool.tile([128, nblk, D], f32, tag="e1")
                    nc.vector.tensor_scalar_add(e1[:], eo, 1.0)
                    att = {}
                    for si in range(2):
                        tprod = apool.tile([128, nblk, D], f32, tag=f"tp{si}")
                        nc.vector.tensor_tensor(
                            tprod[:], ps[si][sl], e1[:],
                            mybir.AluOpType.mult)
                        red = apool.tile([128, nblk], f32, tag=f"red{si}")
                        nc.vector.tensor_reduce(
                            red[:], tprod[:], mybir.AxisListType.X,
                            mybir.AluOpType.add)
                        a = apool.tile([128, nblk], f32, tag=f"att{si}")
                        nc.scalar.activation(
                            a[:], red[:], mybir.ActivationFunctionType.Exp,
                            scale=1.0 / D)
                        att[si] = a
                    den = apool.tile([128, nblk], f32, tag="den")
                    nc.vector.tensor_add(den[:], att[0][:], att[1][:])
                    rec = apool.tile([128, nblk], f32, tag="rec")
                    nc.vector.reciprocal(rec[:], den[:])
                    w0 = apool.tile([128, nblk], f32, tag="w0")
                    nc.vector.tensor_mul(w0[:], att[0][:], rec[:])
                    w1 = apool.tile([128, nblk], f32, tag="w1")
                    nc.vector.tensor_mul(w1[:], att[1][:], rec[:])
                    t0 = apool.tile([128, nblk, D], f32, tag="t0")
                    nc.vector.tensor_tensor(
                        t0[:], ps[0][sl],
                        w0[:].unsqueeze(2).to_broadcast([128, nblk, D]),
                        mybir.AluOpType.mult)
                    new = apool.tile([128, nblk, D], f32, tag="new")
                    nc.vector.tensor_tensor(
                        new[:], ps[1][sl],
                        w1[:].unsqueeze(2).to_broadcast([128, nblk, D]),
                        mybir.AluOpType.mult)
                    nc.vector.tensor_add(new[:], new[:], t0[:])

                    lsl = light[:, blocks[0]:blocks[0] + nblk, :]
                    if layer == 0:
                        # light = emb0 + new
                        nc.vector.tensor_add(lsl, eo, new[:])
                    else:
                        nc.vector.tensor_add(lsl, lsl, new[:])
                    if layer == LAYERS - 1:
                        fin = apool.tile([128, nblk, D], f32, tag="fin")
                        nc.vector.tensor_scalar_mul(
                            fin[:], lsl, 1.0 / (LAYERS + 1))
                        nc.sync.dma_start(
                            light_out[:].rearrange(
                                "(b p) d -> p b d", p=128)[
                                :, blocks[0]:blocks[0] + nblk, :],
                            fin[:])

                    if layer < LAYERS - 1:
                        # update own rows + stage for all-gather
                        nc.vector.tensor_copy(eo, new[:])
                        nc.sync.dma_start(
                            stg[:].rearrange("(b p) d -> p b d", p=128)[
                                :, blocks[0]:blocks[0] + nblk, :],
                            new[:])
                        if st == NSTILES - 1:
                            nc.gpsimd.collective_compute(
                                "AllGather", mybir.AluOpType.bypass,
                                ins=[stg.opt()],
                                outs=[tables[layer + 1].opt()],
                                replica_groups=[list(range(NCORES))])
    nc.compile()
    return nc


# ---------------------------------------------------------------- entry
def _get_compiled(inputs):
    key = "module"
    if key in _CACHE:
        return _CACHE[key]
    n_mm, meta, streams = _preprocess(
        np.asarray(inputs["graph_src"]), np.asarray(inputs["graph_dst"]),
        np.asarray(inputs["graph_val"]),
        np.asarray(inputs["sim_src"]), np.asarray(inputs["sim_dst"]),
        np.asarray(inputs["sim_val"]))
    tot16 = sum(L // 16 for (_, _, _, L) in meta)
    tot128 = sum(L // 128 for (_, _, _, L) in meta)
    nc = _build_module(n_mm, meta, tot16, tot128)
    _CACHE[key] = (nc, n_mm, meta, streams)
    return _CACHE[key]


def kernel(user_emb, item_emb, graph_src, graph_dst, graph_val,
           sim_src, sim_dst, sim_val, users, items):
    from concourse.bass_utils import run_bass_kernel_spmd
    import concourse.bass_utils as _bu
    trace = bool(int(os.environ.get("BIGN_TRACE", "0")))
    if trace:
        _bu.upload_artifacts = lambda tmpdir: tmpdir

    inputs = dict(user_emb=user_emb, item_emb=item_emb,
                  graph_src=graph_src, graph_dst=graph_dst,
                  graph_val=graph_val, sim_src=sim_src, sim_dst=sim_dst,
                  sim_val=sim_val, users=users, items=items)
    nc, n_mm, meta, streams = _get_compiled(inputs)

    emb0 = np.concatenate([np.asarray(user_emb, np.float32),
                           np.asarray(item_emb, np.float32)], axis=0)
    in_maps = []
    for c in range(NCORES):
        sl = np.zeros((NPC, D), np.float32)
        sl[:NODES_PER_CORE_REAL] = emb0[c * NODES_PER_CORE_REAL:
                                        (c + 1) * NODES_PER_CORE_REAL]
        in_maps.append(dict(emb_slice=sl, gidx=streams[c]["gidx"],
                            val=streams[c]["val"], dpos=streams[c]["dpos"]))

    res = run_bass_kernel_spmd(nc, in_maps, core_ids=list(range(NCORES)),
                               trace=trace)
    if trace and res.exec_time_ns is not None:
        kernel.last_exec_time_ns = res.exec_time_ns
        kernel.last_trace = res.instructions_and_trace

    light = np.zeros((N, D), np.float32)
    for c in range(NCORES):
        light[c * NODES_PER_CORE_REAL:(c + 1) * NODES_PER_CORE_REAL] = \
            res.results[c]["light_out"][:NODES_PER_CORE_REAL]
    ue = light[:N_USER][np.asarray(users)]
    ie = light[N_USER:][np.asarray(items)]
    return (ue * ie).sum(axis=1).astype(np.float32)



# revision 2
# speedup vs baseline: 2.1048x; 2.1048x over previous
"""BiGN (2-relation LightGCN-style GNN) on 8 Trainium2 NeuronCores.

Strategy (dst-sharded, SPMD), v2:
- Node space padded to 8 x 18816 rows; core k owns rows [k*18816, (k+1)*18816).
  Distinct batch nodes (users/items actually read at the end) are remapped to
  the FIRST 1024 rows (8 blocks) of each core; remaining nodes are dealt
  round-robin by in-degree so per-(block, chunk) edge counts balance across
  cores (shrinks the shared max-over-cores padding).
- Per-layer edge filtering: layer 3 only needs dst in the batch set (1
  supertile instead of 19); layer 2 only needs dst nodes whose output feeds a
  kept layer-3 edge (~82%).
- Both relations share one dma_gather call per (supertile, chunk); dpos codes
  si*128+pos so a single one-hot tile serves both PSUM accumulators.
- Layer-0 table comes directly from a host-provided full-table input (no
  initial AllGather); later tables are AllGather'd in f32 as before.
- The gather index stream is written only to the 32 SBUF partitions the
  consuming SWDGE queue pair actually reads (2 replicas instead of 8).
"""

import os
import numpy as np
import ml_dtypes

# ---------------------------------------------------------------- constants
NCORES = 8
N_USER = 100000
N_ITEM = 50000
N = N_USER + N_ITEM
D = 64
LAYERS = 3
BLOCKS_PER_CORE = 147
NPC = BLOCKS_PER_CORE * 128              # 18816
NPAD = NCORES * NPC                      # 150528
CHUNK_BOUNDS = [0, 32768, 65536, 98304, 131072, NPAD]
NCHUNKS = 5
STILE_BLOCKS = 8
NSTILES = (BLOCKS_PER_CORE + STILE_BLOCKS - 1) // STILE_BLOCKS  # 19
BATCH_BLOCKS = 8                         # first 8 blocks/core hold batch nodes
DEAD_DPOS = 300.0
BF16 = ml_dtypes.bfloat16

_CACHE = {}


# ---------------------------------------------------------------- host prep
def _build_perm(batch_nodes, deg):
    """node -> padded global row (core*NPC + pos). Batch nodes fill pos
    0..~992 of each core; the rest are dealt by degree across (round, block,
    core) so each (core, block) gets a similar degree mix."""
    perm = np.full(N, -1, np.int64)
    order = batch_nodes[np.argsort(-deg[batch_nodes], kind="stable")]
    i = np.arange(len(order))
    perm[order] = (i % NCORES) * NPC + i // NCORES
    nb = np.zeros(NCORES, np.int64)
    np.add.at(nb, i % NCORES, 1)

    nonbatch = np.ones(N, bool)
    nonbatch[batch_nodes] = False
    nb_nodes = np.nonzero(nonbatch)[0]
    nb_order = nb_nodes[np.argsort(-deg[nb_nodes], kind="stable")]

    # slot sequence: r (0..127) x block (0..146) x core (0..7)
    r = np.repeat(np.arange(128), BLOCKS_PER_CORE * NCORES)
    b = np.tile(np.repeat(np.arange(BLOCKS_PER_CORE), NCORES), 128)
    c = np.tile(np.arange(NCORES), 128 * BLOCKS_PER_CORE)
    pos = b * 128 + r
    ok = pos >= nb[c]          # skip batch-occupied prefix slots
    pos, c = pos[ok], c[ok]
    take = len(nb_order)
    perm[nb_order] = c[:take] * NPC + pos[:take]
    return perm


def _preprocess(graph_src, graph_dst, graph_val, sim_src, sim_dst, sim_val,
                users, items):
    gs, gd = graph_src.astype(np.int64), graph_dst.astype(np.int64)
    ss, sd = sim_src.astype(np.int64), sim_dst.astype(np.int64)
    batch_nodes = np.unique(np.concatenate([users, N_USER + np.asarray(items)]))
    inB = np.zeros(N, bool)
    inB[batch_nodes] = True
    k3 = (inB[gd], inB[sd])
    S3 = np.zeros(N, bool)
    S3[gs[k3[0]]] = True
    S3[ss[k3[1]]] = True
    D2 = S3 | inB
    k2 = (D2[gd], D2[sd])
    ones = np.ones(len(gs), bool)
    layer_masks = [(ones, ones), k2, k3]

    deg = np.zeros(N, np.int64)
    np.add.at(deg, gd, 1)
    np.add.at(deg, sd, 1)
    perm = _build_perm(batch_nodes, deg)

    # per (layer, si): sorted edge fields
    cb = np.asarray(CHUNK_BOUNDS)
    per = {}
    counts = np.zeros((NCORES, LAYERS, NCHUNKS, BLOCKS_PER_CORE, 2), np.int64)
    for l in range(LAYERS):
        for si, (s_, d_, v_) in enumerate(
                ((gs, gd, graph_val), (ss, sd, sim_val))):
            m = layer_masks[l][si]
            sp = perm[s_[m]]
            dp = perm[d_[m]]
            vv = np.asarray(v_)[m]
            core = dp // NPC
            rem = dp % NPC
            blk = rem // 128
            dpos = rem % 128
            ch = np.searchsorted(cb[1:], sp, side="right")
            lsrc = sp - cb[ch]
            if l == LAYERS - 1:
                assert blk.max() < BATCH_BLOCKS
            key = ((core * NCHUNKS + ch) * BLOCKS_PER_CORE + blk)
            order = np.argsort(key, kind="stable")
            ksort = key[order]
            edges_cells = np.arange(NCORES * NCHUNKS * BLOCKS_PER_CORE)
            starts = np.searchsorted(ksort, edges_cells)
            ends = np.searchsorted(ksort, edges_cells + 1)
            per[(l, si)] = (lsrc[order], dpos[order], vv[order],
                            starts, ends)
            cnt = (ends - starts).reshape(NCORES, NCHUNKS, BLOCKS_PER_CORE)
            counts[:, l, :, :, si] = cnt
    nmax = counts.max(axis=0)  # [LAYERS, NCHUNKS, BLOCKS, 2]

    # ---- shared call/run layout
    meta = []   # (l, st, ch, Lc, o16, o128, runs)
    o16 = o128 = 0
    for l in range(LAYERS):
        nst = NSTILES if l < LAYERS - 1 else 1
        for st in range(nst):
            blocks = list(range(st * STILE_BLOCKS,
                                min((st + 1) * STILE_BLOCKS, BLOCKS_PER_CORE)))
            for ch in range(NCHUNKS):
                runs = []
                cum = 0
                last_col_si = {}
                for b in blocks:
                    for si in range(2):
                        n = int(nmax[l, ch, b, si])
                        if n == 0:
                            continue
                        col = cum // 128
                        # avoid two same-si runs sharing a column
                        if last_col_si.get(col) == si:
                            cum = (col + 1) * 128
                        runs.append((b - blocks[0], si, cum, cum + n))
                        cum += n
                        last = (cum - 1) // 128
                        last_col_si = {last: si}
                Lc = max(128, -(-cum // 128) * 128)
                meta.append((l, st, ch, Lc, o16, o128, runs))
                o16 += Lc // 16
                o128 += Lc // 128
    tot16, tot128 = o16, o128

    # ---- per-core stream fill
    streams = []
    for c in range(NCORES):
        gi = np.zeros(tot128 * 128, np.int16)
        va = np.zeros(tot128 * 128, np.float32)
        dz = np.full(tot128 * 128, DEAD_DPOS, np.float32)
        for (l, st, ch, Lc, mo16, mo128, runs) in meta:
            base = mo128 * 128
            b0 = st * STILE_BLOCKS
            for (bl, si, slot_lo, slot_hi) in runs:
                lsrc, dpos, vv, starts, ends = per[(l, si)]
                k = ((c * NCHUNKS + ch) * BLOCKS_PER_CORE + (b0 + bl))
                s, e = starts[k], ends[k]
                n = e - s
                assert n <= slot_hi - slot_lo
                gi[base + slot_lo: base + slot_lo + n] = lsrc[s:e]
                va[base + slot_lo: base + slot_lo + n] = vv[s:e]
                dz[base + slot_lo: base + slot_lo + n] = dpos[s:e] + 128 * si
        # per-call reshape into stream tensors
        gi_parts, va_parts, dz_parts = [], [], []
        for (l, st, ch, Lc, mo16, mo128, runs) in meta:
            base = mo128 * 128
            g = gi[base: base + Lc]
            gw = np.ascontiguousarray(g.reshape(-1, 16).T)     # [16, Lc/16]
            gi_parts.append(np.tile(gw, (2, 1)))               # [32, Lc/16]
            va_parts.append(np.ascontiguousarray(
                va[base: base + Lc].reshape(-1, 128).T))       # [128, Lc/128]
            dz_parts.append(np.ascontiguousarray(
                dz[base: base + Lc].reshape(-1, 128).T))
        streams.append(dict(
            gidx=np.concatenate(gi_parts, axis=1),
            val=np.concatenate(va_parts, axis=1),
            dpos=np.concatenate(dz_parts, axis=1).astype(BF16),
        ))
    return perm, meta, tot16, tot128, streams


# ---------------------------------------------------------------- device
def _build_module(meta, tot16, tot128):
    import concourse.bacc as bacc
    import concourse.mybir as mybir
    import concourse.tile as tile
    from concourse.library_config import mlp

    f32 = mybir.dt.float32
    bf16 = mybir.dt.bfloat16

    nc = bacc.Bacc("TRN2", target_bir_lowering=False, debug=False,
                   num_devices=NCORES, num_swdge_queues=4)
    emb_full = nc.dram_tensor("emb_full", [NPAD, D], f32,
                              kind="ExternalInput")
    emb_slice = nc.dram_tensor("emb_slice", [NPC, D], f32,
                               kind="ExternalInput")
    gidx = nc.dram_tensor("gidx", [32, tot16], mybir.dt.int16,
                          kind="ExternalInput")
    val_in = nc.dram_tensor("val", [128, tot128], f32, kind="ExternalInput")
    dpos_in = nc.dram_tensor("dpos", [128, tot128], bf16,
                             kind="ExternalInput")
    light_out = nc.dram_tensor("light_out", [BATCH_BLOCKS * 128, D], f32,
                               kind="ExternalOutput")

    call_of = {}
    for (l, st, ch, Lc, o16, o128, runs) in meta:
        call_of[(l, st, ch)] = (Lc, o16, o128, runs)

    with tile.TileContext(nc) as tc:
        nc.gpsimd.load_library(mlp)
        with (
            tc.tile_pool(name="persist", bufs=1) as pers,
            tc.tile_pool(name="gath", bufs=3) as gpool,
            tc.tile_pool(name="msgs", bufs=2) as mpool,
            tc.tile_pool(name="ohp", bufs=2) as opool,
            tc.tile_pool(name="small", bufs=6) as spool,
            tc.tile_pool(name="att", bufs=2) as apool,
            tc.tile_pool(name="psum", bufs=4, space="PSUM") as ppool,
            tc.tile_pool(name="dram", bufs=1, space="DRAM") as dram,
            tc.tile_pool(name="dstage", bufs=2, space="DRAM") as dstage,
        ):
            iota0 = pers.tile([128, 128], bf16)
            nc.gpsimd.iota(iota0[:], [[1, 128]], channel_multiplier=0,
                           allow_small_or_imprecise_dtypes=True)
            iota1 = pers.tile([128, 128], bf16)
            nc.vector.tensor_scalar_add(iota1[:], iota0[:], 128.0)
            light = pers.tile([128, BATCH_BLOCKS, D], f32)
            emb_own = pers.tile([128, BLOCKS_PER_CORE, D], f32)
            nc.sync.dma_start(
                emb_own[:],
                emb_slice[:].rearrange("(b p) d -> p b d", p=128))

            tables = [None] + [dram.tile([NPAD, D], f32, addr_space="Shared",
                                         name=f"table{i}")
                               for i in (1, 2)]

            gather_q = 0
            for l in range(LAYERS):
                src_table = emb_full if l == 0 else tables[l]
                nst = NSTILES if l < LAYERS - 1 else 1
                stg = None
                if l < LAYERS - 1:
                    stg = dstage.tile([NPC, D], f32, tag="stage",
                                      name=f"stage{l}")
                for st in range(nst):
                    blocks = list(range(
                        st * STILE_BLOCKS,
                        min((st + 1) * STILE_BLOCKS, BLOCKS_PER_CORE)))
                    nblk = len(blocks)
                    ps = {}
                    for si in range(2):
                        ps[si] = ppool.tile([128, STILE_BLOCKS, D], f32,
                                            tag=f"ps{si}", name=f"ps{si}")
                        nc.vector.memset(ps[si][:], 0.0)
                    for ch in range(NCHUNKS):
                        Lc, o16, o128, runs = call_of[(l, st, ch)]
                        M = Lc // 128
                        q = gather_q
                        gather_q = (gather_q + 1) % 4
                        it = spool.tile([128, Lc // 16], mybir.dt.int16,
                                        tag="gidx")
                        nc.sync.dma_start(
                            it[32 * q:32 * q + 32, :],
                            gidx[:, o16:o16 + Lc // 16])
                        gt = gpool.tile([128, M, D], f32, tag="gath")
                        cbase = CHUNK_BOUNDS[ch]
                        cend = CHUNK_BOUNDS[ch + 1]
                        nc.gpsimd.dma_gather(
                            gt[:], src_table[cbase:cend, :], it[:],
                            Lc, Lc, D, single_packet=False,
                            queue_num=q)
                        vt = spool.tile([128, M], f32, tag="val")
                        nc.sync.dma_start(
                            vt[:], val_in[:, o128:o128 + M])
                        dt = spool.tile([128, M], bf16, tag="dpos")
                        nc.sync.dma_start(
                            dt[:], dpos_in[:, o128:o128 + M])
                        ms = mpool.tile([128, M, D], bf16, tag="msgs")
                        nc.vector.tensor_tensor(
                            ms[:], gt[:],
                            vt[:].unsqueeze(2).to_broadcast([128, M, D]),
                            mybir.AluOpType.mult)
                        oh = opool.tile([128, M, 128], bf16, tag="oh")
                        for (bl, si, slot_lo, slot_hi) in runs:
                            col_lo = slot_lo // 128
                            col_hi = -(-slot_hi // 128)
                            w = col_hi - col_lo
                            io = iota1 if si else iota0
                            nc.vector.tensor_tensor(
                                oh[:, col_lo:col_hi, :],
                                dt[:, col_lo:col_hi].unsqueeze(2)
                                    .to_broadcast([128, w, 128]),
                                io[:].unsqueeze(1).to_broadcast([128, w, 128]),
                                mybir.AluOpType.is_equal)
                            for m in range(col_lo, col_hi):
                                nc.tensor.matmul(
                                    ps[si][:, bl, :],
                                    oh[:, m, :], ms[:, m, :],
                                    start=False, stop=False,
                                    skip_group_check=True)

                    # ---- attention / combine for this super-tile
                    sl = (slice(None), slice(0, nblk), slice(None))
                    eo = emb_own[:, blocks[0]:blocks[0] + nblk, :]
                    e1 = apool.tile([128, nblk, D], f32, tag="e1")
                    nc.vector.tensor_scalar_add(e1[:], eo, 1.0)
                    att = {}
                    for si in range(2):
                        tprod = apool.tile([128, nblk, D], f32, tag=f"tp{si}")
                        nc.vector.tensor_tensor(
                            tprod[:], ps[si][sl], e1[:],
                            mybir.AluOpType.mult)
                        red = apool.tile([128, nblk], f32, tag=f"red{si}")
                        nc.vector.tensor_reduce(
                            red[:], tprod[:], mybir.AxisListType.X,
                            mybir.AluOpType.add)
                        a = apool.tile([128, nblk], f32, tag=f"att{si}")
                        nc.scalar.activation(
                            a[:], red[:], mybir.ActivationFunctionType.Exp,
                            scale=1.0 / D)
                        att[si] = a
                    den = apool.tile([128, nblk], f32, tag="den")
                    nc.vector.tensor_add(den[:], att[0][:], att[1][:])
                    rec = apool.tile([128, nblk], f32, tag="rec")
                    nc.vector.reciprocal(rec[:], den[:])
                    w0 = apool.tile([128, nblk], f32, tag="w0")
                    nc.vector.tensor_mul(w0[:], att[0][:], rec[:])
                    w1 = apool.tile([128, nblk], f32, tag="w1")
                    nc.vector.tensor_mul(w1[:], att[1][:], rec[:])
                    t0 = apool.tile([128, nblk, D], f32, tag="t0")
                    nc.vector.tensor_tensor(
                        t0[:], ps[0][sl],
                        w0[:].unsqueeze(2).to_broadcast([128, nblk, D]),
                        mybir.AluOpType.mult)
                    new = apool.tile([128, nblk, D], f32, tag="new")
                    nc.vector.tensor_tensor(
                        new[:], ps[1][sl],
                        w1[:].unsqueeze(2).to_broadcast([128, nblk, D]),
                        mybir.AluOpType.mult)
                    nc.vector.tensor_add(new[:], new[:], t0[:])

                    if st == 0:
                        lsl = light[:, 0:nblk, :]
                        if l == 0:
                            nc.vector.tensor_add(lsl, eo, new[:])
                        else:
                            nc.vector.tensor_add(lsl, lsl, new[:])
                        if l == LAYERS - 1:
                            fin = apool.tile([128, nblk, D], f32, tag="fin")
                            nc.vector.tensor_scalar_mul(
                                fin[:], lsl, 1.0 / (LAYERS + 1))
                            nc.sync.dma_start(
                                light_out[:].rearrange(
                                    "(b p) d -> p b d", p=128),
                                fin[:])

                    if l < LAYERS - 1:
                        nc.vector.tensor_copy(eo, new[:])
                        nc.sync.dma_start(
                            stg[:].rearrange("(b p) d -> p b d", p=128)[
                                :, blocks[0]:blocks[0] + nblk, :],
                            new[:])
                        if st == NSTILES - 1:
                            nc.gpsimd.collective_compute(
                                "AllGather", mybir.AluOpType.bypass,
                                ins=[stg.opt()],
                                outs=[tables[l + 1].opt()],
                                replica_groups=[list(range(NCORES))])
    nc.compile()
    return nc


# ---------------------------------------------------------------- entry
def _get_compiled(inputs):
    key = "module"
    if key in _CACHE:
        return _CACHE[key]
    perm, meta, tot16, tot128, streams = _preprocess(
        np.asarray(inputs["graph_src"]), np.asarray(inputs["graph_dst"]),
        np.asarray(inputs["graph_val"]),
        np.asarray(inputs["sim_src"]), np.asarray(inputs["sim_dst"]),
        np.asarray(inputs["sim_val"]),
        np.asarray(inputs["users"]), np.asarray(inputs["items"]))
    nc = _build_module(meta, tot16, tot128)
    _CACHE[key] = (nc, perm, meta, streams)
    return _CACHE[key]


def kernel(user_emb, item_emb, graph_src, graph_dst, graph_val,
           sim_src, sim_dst, sim_val, users, items):
    from concourse.bass_utils import run_bass_kernel_spmd
    import concourse.bass_utils as _bu
    trace = bool(int(os.environ.get("BIGN_TRACE", "0")))
    if trace:
        _bu.upload_artifacts = lambda tmpdir: tmpdir

    inputs = dict(user_emb=user_emb, item_emb=item_emb,
                  graph_src=graph_src, graph_dst=graph_dst,
                  graph_val=graph_val, sim_src=sim_src, sim_dst=sim_dst,
                  sim_val=sim_val, users=users, items=items)
    nc, perm, meta, streams = _get_compiled(inputs)

    emb0 = np.concatenate([np.asarray(user_emb, np.float32),
                           np.asarray(item_emb, np.float32)], axis=0)
    emb_pad = np.zeros((NPAD, D), np.float32)
    emb_pad[perm] = emb0
    in_maps = []
    for c in range(NCORES):
        in_maps.append(dict(
            emb_full=emb_pad,
            emb_slice=np.ascontiguousarray(emb_pad[c * NPC:(c + 1) * NPC]),
            gidx=streams[c]["gidx"], val=streams[c]["val"],
            dpos=streams[c]["dpos"]))

    res = run_bass_kernel_spmd(nc, in_maps, core_ids=list(range(NCORES)),
                               trace=trace)
    if trace and res.exec_time_ns is not None:
        kernel.last_exec_time_ns = res.exec_time_ns
        kernel.last_trace = res.instructions_and_trace

    # light values for batch rows (first 1024 rows of each core)
    light_batch = np.stack([res.results[c]["light_out"]
                            for c in range(NCORES)])   # [8, 1024, D]
    g_users = perm[np.asarray(users)]
    g_items = perm[N_USER + np.asarray(items)]
    ue = light_batch[g_users // NPC, g_users % NPC]
    ie = light_batch[g_items // NPC, g_items % NPC]
    return (ue * ie).sum(axis=1).astype(np.float32)
